# Initial kernel scaffold
#
"""Trainium2 Bass kernel for nn_BinaryTemporalBlock (Conv-TasNet-style binary
temporal block): 1x1 binarized conv (128->512) -> gLN -> PReLU -> dilated
depthwise binarized conv (K=3, dil=4) -> gLN -> PReLU -> two 1x1 binarized
convs (512->128 residual-out and 512->128 skip).

Sharding: data-parallel over batch. B=8 samples on 8 NeuronCores, one sample
per core; gLN is per-sample so no collectives are needed.

Device strategy per core (sample = [C=128, T=4000]):
  - Weights are XNOR-binarized on the host: sign matrices (exact +-1 in bf16)
    go to the PE array; the per-output-channel alpha scales stay fp32 and are
    folded into the PSUM->SBUF drain (ScalarE activation scale / DVE
    tensor_scalar), so matmul results are exact sums of bf16 activations.
  - gLN statistics: per-partition sums fused into the drains via accum_out;
    sum of squares via DVE tensor_tensor_reduce; partition reduction via
    GPSIMD partition_all_reduce (leaves totals on all partitions).
  - norm+PReLU: DVE tensor_scalar (affine, 4x bf16) + scalar_tensor_tensor
    max(p*z, z) (valid for p <= 1).
  - depthwise dilated conv: 3 diagonal-sign matmuls per tile accumulating in
    PSUM on the PE (taps at t-4, t, t+4 via shifted access patterns on a
    halo-padded tile).
"""

import os
import sys

sys.path.insert(0, "/opt/trn_rl_repo")

import numpy as np
import ml_dtypes

import concourse.bass as bass
import concourse.tile as tile
from concourse import mybir
from concourse.bass_isa import ReduceOp
from concourse.bass_utils import run_bass_kernel_spmd

F32 = mybir.dt.float32
BF16 = mybir.dt.bfloat16
NPBF16 = ml_dtypes.bfloat16
ALU = mybir.AluOpType
AFT = mybir.ActivationFunctionType
AX = mybir.AxisListType

B, C, H, SC, T = 8, 128, 512, 128, 4000
HR = H // 128          # 4 h-rows of 128 partitions
CW = 500               # chunk width (one PSUM bank of fp32)
NCH = T // CW          # 8 chunks
DIL = 4
EPS = 1e-8
NTOT = float(H * T)    # gLN normalizer

# tunables
CFG = {
    "act_drain1": 6,   # chunks per row drained on ScalarE (rest on DVE)
    "act_drain2": 6,
    "np_half": 2000,   # np1 op width
}

# stash for test.py (exec time etc.)
last_run_info = {}


def _binarize(w):
    alpha = np.mean(np.abs(w), axis=tuple(range(1, w.ndim)))
    return alpha.astype(np.float32), np.sign(w).astype(np.float32)


def _cols(v):
    """[512] channel vector -> [128, HR] column-per-h-row layout."""
    return np.ascontiguousarray(v.reshape(HR, 128).T.astype(np.float32))


def _prep(inputs):
    x = np.asarray(inputs["x"], np.float32)
    p1 = float(np.asarray(inputs["p1"]))
    p2 = float(np.asarray(inputs["p2"]))
    b1 = np.asarray(inputs["b1"], np.float32).reshape(-1)
    g1 = np.asarray(inputs["g1"], np.float32).reshape(-1)
    be1 = np.asarray(inputs["be1"], np.float32).reshape(-1)
    g2 = np.asarray(inputs["g2"], np.float32).reshape(-1)
    be2 = np.asarray(inputs["be2"], np.float32).reshape(-1)
    b2 = np.asarray(inputs["b2"], np.float32).reshape(-1)
    bsk = np.asarray(inputs["b_skip"], np.float32).reshape(-1)

    a1, s1 = _binarize(np.asarray(inputs["w1"], np.float32))      # [512],[512,128,1]
    adw, sdw = _binarize(np.asarray(inputs["w_dw"], np.float32))  # [512],[512,1,3]
    a2, s2 = _binarize(np.asarray(inputs["w2"], np.float32))      # [128],[128,512,1]
    ask, ssk = _binarize(np.asarray(inputs["w_skip"], np.float32))
    s1 = s1[:, :, 0]      # [512,128]
    sdw = sdw[:, 0, :]    # [512,3]
    s2 = s2[:, :, 0]      # [128,512]
    ssk = ssk[:, :, 0]

    lhsT1 = np.ascontiguousarray(s1.T.astype(NPBF16))             # [128,512]
    dwdiag = np.zeros((128, HR * 3, 128), NPBF16)
    for r in range(HR):
        for k in range(3):
            np.fill_diagonal(dwdiag[:, r * 3 + k, :], sdw[r * 128:(r + 1) * 128, k])
    lhsT2 = np.zeros((128, HR, 128), NPBF16)
    lhsTsk = np.zeros((128, HR, 128), NPBF16)
    for k in range(HR):
        lhsT2[:, k, :] = s2[:, k * 128:(k + 1) * 128].T
        lhsTsk[:, k, :] = ssk[:, k * 128:(k + 1) * 128].T

    # per-h-row parameter columns [128, HR]
    hpar = np.stack([_cols(a1), _cols(b1), _cols(g1), _cols(be1),
                     _cols(adw), _cols(g2), _cols(be2)], axis=1)  # [128,7,HR]
    hpar = np.ascontiguousarray(hpar.reshape(128, 7 * HR))
    # per-C columns [128, 4]: a2, b2, ask, bsk
    cpar = np.ascontiguousarray(
        np.stack([a2, b2, ask, bsk], axis=1).astype(np.float32))  # [128,4]

    common = {
        "lhsT1": lhsT1,
        "dwdiag": np.ascontiguousarray(dwdiag.reshape(128, HR * 3 * 128)),
        "lhsT2": np.ascontiguousarray(lhsT2.reshape(128, HR * 128)),
        "lhsTsk": np.ascontiguousarray(lhsTsk.reshape(128, HR * 128)),
        "hpar": hpar,
        "cpar": cpar,
    }
    return x, p1, p2, common


def _stats_join(nc, pools, parts_list, scb, gcol, becol, ws, eps_t):
    """Combine per-partition partial sums -> scale/bias columns.

    parts_list: [(tile_ap, ncols_sum), (tile_ap_sq, ncols_sq)] where the first
    entries are sums, last is sum-of-squares partials.
    scb: [128, 2*HR] output (cols 0:HR scale, HR:2*HR bias).
    ws: [128, 8] fp32 workspace. layout: 0 sum,1 sumsq,2 m,3 E2,4 mneg,
    5 var,6 sd,7 rs
    """
    sum_parts, sq_parts = parts_list
    # reduce partials along free dim
    if len(sum_parts) == 1:
        nc.vector.reduce_sum(out=ws[:, 0:1], in_=sum_parts[0], axis=AX.X)
    else:
        tmp = pools["small"].tile([128, 2], F32, tag="join_tmp")
        nc.vector.reduce_sum(out=tmp[:, 0:1], in_=sum_parts[0], axis=AX.X)
        nc.vector.reduce_sum(out=tmp[:, 1:2], in_=sum_parts[1], axis=AX.X)
        nc.vector.reduce_sum(out=ws[:, 0:1], in_=tmp[:, 0:2], axis=AX.X)
    nc.vector.reduce_sum(out=ws[:, 1:2], in_=sq_parts, axis=AX.X)
    # total over partitions, broadcast back to every partition
    nc.gpsimd.partition_all_reduce(ws[:, 0:2], ws[:, 0:2], 128, ReduceOp.add)
    # m = S/N ; E2 = Q/N ; mneg = -m
    nc.scalar.activation(ws[:, 2:4], ws[:, 0:2], AFT.Identity, scale=1.0 / NTOT)
    nc.scalar.activation(ws[:, 4:5], ws[:, 0:1], AFT.Identity, scale=-1.0 / NTOT)
    # var = E2 - m^2
    nc.vector.tensor_tensor(out=ws[:, 5:6], in0=ws[:, 2:3], in1=ws[:, 2:3],
                            op=ALU.mult)
    nc.vector.tensor_tensor(out=ws[:, 5:6], in0=ws[:, 3:4], in1=ws[:, 5:6],
                            op=ALU.subtract)
    # rs = 1/sqrt(var+eps)
    nc.scalar.activation(ws[:, 6:7], ws[:, 5:6], AFT.Sqrt, bias=eps_t[:, 0:1])
    nc.vector.reciprocal(ws[:, 7:8], ws[:, 6:7])
    # scale = g*rs ; bias = be + mneg*scale
    nc.vector.tensor_scalar_mul(out=scb[:, 0:HR], in0=gcol, scalar1=ws[:, 7:8])
    nc.vector.scalar_tensor_tensor(out=scb[:, HR:2 * HR], in0=scb[:, 0:HR],
                                   scalar=ws[:, 4:5], in1=becol,
                                   op0=ALU.mult, op1=ALU.add)


def _build(p1, p2):
    nc = bass.Bass()
    x_in = nc.declare_dram_parameter("x_in", [C, T], F32, False)
    lhsT1_in = nc.declare_dram_parameter("lhsT1", [128, H], BF16, False)
    dwdiag_in = nc.declare_dram_parameter("dwdiag", [128, HR * 3 * 128], BF16, False)
    lhsT2_in = nc.declare_dram_parameter("lhsT2", [128, HR * 128], BF16, False)
    lhsTsk_in = nc.declare_dram_parameter("lhsTsk", [128, HR * 128], BF16, False)
    hpar_in = nc.declare_dram_parameter("hpar", [128, 7 * HR], F32, False)
    cpar_in = nc.declare_dram_parameter("cpar", [128, 4], F32, False)
    out_r = nc.declare_dram_parameter("out_r", [C, T], F32, True)
    skip_r = nc.declare_dram_parameter("skip_r", [SC, T], F32, True)

    with tile.TileContext(nc) as tc:
        with (
            tc.tile_pool(name="persist", bufs=1) as pp,
            tc.tile_pool(name="outp", bufs=3) as outp,
            tc.tile_pool(name="small", bufs=1) as small,
            tc.tile_pool(name="mm", bufs=7, space="PSUM") as mmp,
        ):
            pools = {"small": small}
            # ---- load weights/params
            w1t = pp.tile([128, H], BF16, tag="w1t")
            nc.sync.dma_start(out=w1t[:], in_=lhsT1_in[:])
            dwd = pp.tile([128, HR * 3, 128], BF16, tag="dwd")
            nc.sync.dma_start(out=dwd[:], in_=dwdiag_in[:].rearrange(
                "p (a b) -> p a b", b=128))
            w2t = pp.tile([128, HR, 128], BF16, tag="w2t")
            nc.sync.dma_start(out=w2t[:], in_=lhsT2_in[:].rearrange(
                "p (a b) -> p a b", b=128))
            wst = pp.tile([128, HR, 128], BF16, tag="wst")
            nc.sync.dma_start(out=wst[:], in_=lhsTsk_in[:].rearrange(
                "p (a b) -> p a b", b=128))
            hpar = pp.tile([128, 7, HR], F32, tag="hpar")
            nc.sync.dma_start(out=hpar[:], in_=hpar_in[:].rearrange(
                "p (a b) -> p a b", b=HR))
            cpar = pp.tile([128, 4], F32, tag="cpar")
            nc.sync.dma_start(out=cpar[:], in_=cpar_in[:])
            a1c, b1c, g1c, be1c, adwc, g2c, be2c = (hpar[:, i, :] for i in range(7))
            a2c, b2c, askc, bskc = (cpar[:, i:i + 1] for i in range(4))
            eps_t = small.tile([128, 1], F32, tag="eps")
            nc.vector.memset(eps_t[:], EPS)

            # ---- big persistent tensors
            x_t = pp.tile([128, T], F32, tag="x")
            xb = pp.tile([128, T], BF16, tag="xb")       # also bf16 scratch later
            h1 = [pp.tile([128, T], BF16, tag=f"h1_{r}") for r in range(HR)]
            h1n = [pp.tile([128, T + 2 * DIL], BF16, tag=f"h1n_{r}") for r in range(HR)]
            h2 = [pp.tile([128, T], BF16, tag=f"h2_{r}") for r in range(HR)]
            h2n = [pp.tile([128, T], BF16, tag=f"h2n_{r}") for r in range(HR)]

            # stats partials
            s1p_act = small.tile([128, HR * CFG["act_drain1"]], F32, tag="s1pa")
            s1p_dve = small.tile([128, HR * (NCH - CFG["act_drain1"])], F32, tag="s1pd")
            sq1p = small.tile([128, HR], F32, tag="sq1p")
            s2p_act = small.tile([128, HR * CFG["act_drain2"]], F32, tag="s2pa")
            s2p_dve = small.tile([128, HR * (NCH - CFG["act_drain2"])], F32, tag="s2pd")
            sq2p = small.tile([128, HR], F32, tag="sq2p")
            scb1 = small.tile([128, 2 * HR], F32, tag="scb1")
            scb2 = small.tile([128, 2 * HR], F32, tag="scb2")
            ws1 = small.tile([128, 8], F32, tag="ws1")
            ws2 = small.tile([128, 8], F32, tag="ws2")

            # ---- stage 0: load x in 4 pieces, cast to bf16, fold b2 into x
            for q in range(4):
                sl = slice(q * 1000, (q + 1) * 1000)
                nc.sync.dma_start(out=x_t[:, sl], in_=x_in[:, sl])
                nc.gpsimd.tensor_copy(out=xb[:, sl], in_=x_t[:, sl])
                nc.gpsimd.tensor_scalar(out=x_t[:, sl], in0=x_t[:, sl],
                                        scalar1=b2c, scalar2=None, op0=ALU.add)

            # ---- stage 1: conv1 matmuls + drains (+ fused row-sums)
            na1 = CFG["act_drain1"]
            for r in range(HR):
                for c in range(NCH):
                    ps = mmp.tile([128, CW], F32, tag="mm")
                    nc.tensor.matmul(ps[:], w1t[:, r * 128:(r + 1) * 128],
                                     xb[:, c * CW:(c + 1) * CW],
                                     start=True, stop=True)
                    csl = slice(c * CW, (c + 1) * CW)
                    if c < na1:
                        i = r * na1 + c
                        nc.scalar.activation(h1[r][:, csl], ps[:], AFT.Identity,
                                             bias=b1c[:, r:r + 1],
                                             scale=a1c[:, r:r + 1],
                                             accum_out=s1p_act[:, i:i + 1])
                    else:
                        i = r * (NCH - na1) + (c - na1)
                        nc.vector.tensor_scalar(out=h1[r][:, csl], in0=ps[:],
                                                scalar1=a1c[:, r:r + 1],
                                                scalar2=b1c[:, r:r + 1],
                                                op0=ALU.mult, op1=ALU.add,
                                                accum_out=s1p_dve[:, i:i + 1])
            # sum of squares of h1 (bf16 from SBUF, 2x mode); xb is dead as
            # conv1 input after the last matmul -> reuse as dummy out target
            for r in range(HR):
                nc.vector.tensor_tensor_reduce(
                    out=xb[:, 0:T], in0=h1[r][:], in1=h1[r][:], scale=1.0,
                    scalar=0.0, op0=ALU.mult, op1=ALU.add,
                    accum_out=sq1p[:, r:r + 1])

            _stats_join(nc, pools, [[s1p_act[:], s1p_dve[:]], sq1p[:]],
                        scb1, g1c, be1c, ws1, eps_t)

            # ---- stage 2a: norm+prelu -> h1n (data at offset DIL, halos zero)
            for r in range(HR):
                nc.gpsimd.memset(h1n[r][:, 0:DIL], 0)
                nc.gpsimd.memset(h1n[r][:, DIL + T:], 0)
                w = CFG["np_half"]
                for hh in range(T // w):
                    sl = slice(hh * w, (hh + 1) * w)
                    z = xb[:, 0:w]
                    nc.vector.tensor_scalar(out=z, in0=h1[r][:, sl],
                                            scalar1=scb1[:, r:r + 1],
                                            scalar2=scb1[:, HR + r:HR + r + 1],
                                            op0=ALU.mult, op1=ALU.add)
                    nc.vector.scalar_tensor_tensor(
                        out=h1n[r][:, DIL + hh * w:DIL + (hh + 1) * w],
                        in0=z, scalar=p1, in1=z, op0=ALU.mult, op1=ALU.max)

            # ---- stage 2b: depthwise dilated conv (3 diag matmuls / tile)
            na2 = CFG["act_drain2"]
            for r in range(HR):
                for half in range(2):
                    pss = [mmp.tile([128, CW], F32, tag="mm") for _ in range(4)]
                    for k in range(3):
                        off = (k - 1) * DIL
                        for c4 in range(4):
                            c = half * 4 + c4
                            st = DIL + c * CW + off
                            nc.tensor.matmul(pss[c4][:],
                                             dwd[:, r * 3 + k, :],
                                             h1n[r][:, st:st + CW],
                                             start=(k == 0), stop=(k == 2))
                    for c4 in range(4):
                        c = half * 4 + c4
                        csl = slice(c * CW, (c + 1) * CW)
                        if c < na2:
                            i = r * na2 + c
                            nc.scalar.activation(h2[r][:, csl], pss[c4][:],
                                                 AFT.Identity, bias=0.0,
                                                 scale=adwc[:, r:r + 1],
                                                 accum_out=s2p_act[:, i:i + 1])
                        else:
                            i = r * (NCH - na2) + (c - na2)
                            nc.vector.tensor_scalar(
                                out=h2[r][:, csl], in0=pss[c4][:],
                                scalar1=adwc[:, r:r + 1], scalar2=None,
                                op0=ALU.mult, accum_out=s2p_dve[:, i:i + 1])
            for r in range(HR):
                nc.vector.tensor_tensor_reduce(
                    out=xb[:, 0:T], in0=h2[r][:], in1=h2[r][:], scale=1.0,
                    scalar=0.0, op0=ALU.mult, op1=ALU.add,
                    accum_out=sq2p[:, r:r + 1])

            _stats_join(nc, pools, [[s2p_act[:], s2p_dve[:]], sq2p[:]],
                        scb2, g2c, be2c, ws2, eps_t)

            # ---- stage 3: norm+prelu -> h2n (chunk-major for final matmuls)
            for c in range(NCH):
                csl = slice(c * CW, (c + 1) * CW)
                for r in range(HR):
                    z = xb[:, 0:CW]
                    nc.vector.tensor_scalar(out=z, in0=h2[r][:, csl],
                                            scalar1=scb2[:, r:r + 1],
                                            scalar2=scb2[:, HR + r:HR + r + 1],
                                            op0=ALU.mult, op1=ALU.add)
                    nc.vector.scalar_tensor_tensor(
                        out=h2n[r][:, csl], in0=z, scalar=p2, in1=z,
                        op0=ALU.mult, op1=ALU.max)

            # ---- final 1x1 convs + residual add + skip
            for pair in range(NCH // 2):
                cs = [2 * pair, 2 * pair + 1]
                ps_o = [mmp.tile([128, CW], F32, tag="mm") for _ in cs]
                ps_s = [mmp.tile([128, CW], F32, tag="mm") for _ in cs]
                for k in range(HR):
                    for j, c in enumerate(cs):
                        csl = slice(c * CW, (c + 1) * CW)
                        nc.tensor.matmul(ps_o[j][:], w2t[:, k, :], h2n[k][:, csl],
                                         start=(k == 0), stop=(k == HR - 1))
                for k in range(HR):
                    for j, c in enumerate(cs):
                        csl = slice(c * CW, (c + 1) * CW)
                        nc.tensor.matmul(ps_s[j][:], wst[:, k, :], h2n[k][:, csl],
                                         start=(k == 0), stop=(k == HR - 1))
                for j, c in enumerate(cs):
                    csl = slice(c * CW, (c + 1) * CW)
                    oc = outp.tile([128, CW], F32, tag="oc")
                    nc.vector.scalar_tensor_tensor(out=oc[:], in0=ps_o[j][:],
                                                   scalar=a2c, in1=x_t[:, csl],
                                                   op0=ALU.mult, op1=ALU.add)
                    nc.sync.dma_start(out=out_r[:, csl], in_=oc[:])
                    sc = outp.tile([128, CW], F32, tag="sc")
                    nc.scalar.activation(sc[:], ps_s[j][:], AFT.Identity,
                                         bias=bskc, scale=askc)
                    nc.sync.dma_start(out=skip_r[:, csl], in_=sc[:])
    return nc


def kernel(**inputs):
    x, p1, p2, common = _prep(inputs)
    nc = _build(p1, p2)
    in_maps = [dict(common, x_in=np.ascontiguousarray(x[b])) for b in range(B)]
    trace = bool(int(os.environ.get("KERNEL_TRACE", "0")))
    res = run_bass_kernel_spmd(nc, in_maps, core_ids=list(range(B)), trace=trace)
    last_run_info.clear()
    last_run_info["exec_time_ns"] = res.exec_time_ns
    last_run_info["results"] = res
    out = np.stack([r["out_r"] for r in res.results]).astype(np.float32)
    skip = np.stack([r["skip_r"] for r in res.results]).astype(np.float32)
    return out, skip


# revision 27
# speedup vs baseline: 1.7025x; 1.7025x over previous
"""Trainium2 Bass kernel for nn_BinaryTemporalBlock (Conv-TasNet-style binary
temporal block): 1x1 binarized conv (128->512) -> gLN -> PReLU -> dilated
depthwise binarized conv (K=3, dil=4) -> gLN -> PReLU -> two 1x1 binarized
convs (512->128 residual-out and 512->128 skip).

Sharding: data-parallel over batch. B=8 samples on 8 NeuronCores, one sample
per core; gLN is per-sample so no collectives are needed.

Device strategy per core (sample = [C=128, T=4000]):
  - Weights are XNOR-binarized on the host: sign matrices (exact +-1 in bf16)
    stream through the PE array; the per-output-channel alpha scales stay fp32
    and are folded into the PSUM->SBUF drains, so matmul outputs are exact
    sums of bf16 activations.
  - Matmuls fill a 4-bank PSUM group ([128,4,512]); one drain instruction
    (ScalarE activation or DVE tensor_scalar, both with per-partition
    scale/bias) empties the whole group and accumulates the per-channel sums
    for the gLN statistics via accum_out.
  - Sum-of-squares: ScalarE Square+accum / DVE scalar_tensor_tensor+accum
    (split across both engines). Partition reduction + broadcast of the
    scalar stats via two tiny PE matmuls against ones vectors.
  - norm+PReLU: single ScalarE Prelu pass (scale/bias/alpha) on some tiles,
    DVE tensor_scalar + scalar_tensor_tensor max(p*z,z) on others (p<=1).
  - depthwise dilated conv: 3 diagonal-sign matmuls per tile accumulating in
    PSUM (taps at t-4, t, t+4 via shifted APs on a halo-padded tile).
"""

import os
import sys

sys.path.insert(0, "/opt/trn_rl_repo")

import numpy as np
import ml_dtypes

import concourse.bass as bass
import concourse.tile as tile
from concourse import bacc
from concourse import mybir
from concourse.bass_utils import run_bass_kernel_spmd

F32 = mybir.dt.float32
BF16 = mybir.dt.bfloat16
NPBF16 = ml_dtypes.bfloat16
ALU = mybir.AluOpType
AFT = mybir.ActivationFunctionType
AX = mybir.AxisListType

B, C, H, SC, T = 8, 128, 512, 128, 4000
HR = H // 128          # 4 h-rows of 128 partitions
CW = 500               # matmul chunk width (<=512 fp32 PSUM bank)
GW = 4 * CW            # drain-group width (4 banks)
NG = T // GW           # 2 groups per row
DIL = 4
EPS = 1e-8
NTOT = float(H * T)

CFG = {
    "drain_act": 5,    # of 8 drain groups per block on ScalarE (rest DVE)
    "np1_act_rows": 3,  # h1n rows written by ScalarE Prelu (rest DVE)
    "np2_act": 9,     # of 16 np2 1000-wide units on ScalarE Prelu
    "sumsq_act": 2,    # of 4 rows per block on ScalarE Square
}

last_run_info = {}


def _binarize(w):
    alpha = np.mean(np.abs(w), axis=tuple(range(1, w.ndim)))
    return alpha.astype(np.float32), np.sign(w).astype(np.float32)


def _cols(v):
    """[512] channel vector -> [128, HR] column-per-h-row layout."""
    return np.ascontiguousarray(v.reshape(HR, 128).T.astype(np.float32))


def _prep(inputs):
    x = np.asarray(inputs["x"], np.float32)
    p1 = float(np.asarray(inputs["p1"]))
    p2 = float(np.asarray(inputs["p2"]))
    b1 = np.asarray(inputs["b1"], np.float32).reshape(-1)
    g1 = np.asarray(inputs["g1"], np.float32).reshape(-1)
    be1 = np.asarray(inputs["be1"], np.float32).reshape(-1)
    g2 = np.asarray(inputs["g2"], np.float32).reshape(-1)
    be2 = np.asarray(inputs["be2"], np.float32).reshape(-1)
    b2 = np.asarray(inputs["b2"], np.float32).reshape(-1)
    bsk = np.asarray(inputs["b_skip"], np.float32).reshape(-1)

    a1, s1 = _binarize(np.asarray(inputs["w1"], np.float32))
    adw, sdw = _binarize(np.asarray(inputs["w_dw"], np.float32))
    a2, s2 = _binarize(np.asarray(inputs["w2"], np.float32))
    ask, ssk = _binarize(np.asarray(inputs["w_skip"], np.float32))
    s1 = s1[:, :, 0]      # [512,128]
    sdw = sdw[:, 0, :]    # [512,3]
    s2 = s2[:, :, 0]      # [128,512]
    ssk = ssk[:, :, 0]

    lhsT1 = np.ascontiguousarray(s1.T.astype(NPBF16))             # [128,512]
    dwdiag = np.zeros((128, HR * 3, 128), NPBF16)
    for r in range(HR):
        for k in range(3):
            np.fill_diagonal(dwdiag[:, r * 3 + k, :], sdw[r * 128:(r + 1) * 128, k])
    lhsT2 = np.zeros((128, HR, 128), NPBF16)
    lhsTsk = np.zeros((128, HR, 128), NPBF16)
    for k in range(HR):
        lhsT2[:, k, :] = s2[:, k * 128:(k + 1) * 128].T
        lhsTsk[:, k, :] = ssk[:, k * 128:(k + 1) * 128].T

    hpar = np.stack([_cols(a1), _cols(b1), _cols(g1), _cols(be1),
                     _cols(adw), _cols(g2), _cols(be2)], axis=1)  # [128,7,HR]
    hpar = np.ascontiguousarray(hpar.reshape(128, 7 * HR))
    cpar = np.ascontiguousarray(
        np.stack([a2, b2, ask, bsk], axis=1).astype(np.float32))  # [128,4]

    common = {
        "lhsT1": lhsT1,
        "dwdiag": np.ascontiguousarray(dwdiag.reshape(128, HR * 3 * 128)),
        "lhsT2": np.ascontiguousarray(lhsT2.reshape(128, HR * 128)),
        "lhsTsk": np.ascontiguousarray(lhsTsk.reshape(128, HR * 128)),
        "hpar": hpar,
        "cpar": cpar,
    }
    return x, p1, p2, common


def _r3(ap, b=CW):
    """[128, k*b] contiguous slice -> [128, k, b] view."""
    return ap.rearrange("p (a b) -> p a b", b=b)


def _build(p1, p2):
    nc = bacc.Bacc("TRN2", target_bir_lowering=False, debug=False, num_devices=8)
    x_in = nc.declare_dram_parameter("x_in", [C, T], F32, False)
    lhsT1_in = nc.declare_dram_parameter("lhsT1", [128, H], BF16, False)
    dwdiag_in = nc.declare_dram_parameter("dwdiag", [128, HR * 3 * 128], BF16, False)
    lhsT2_in = nc.declare_dram_parameter("lhsT2", [128, HR * 128], BF16, False)
    lhsTsk_in = nc.declare_dram_parameter("lhsTsk", [128, HR * 128], BF16, False)
    hpar_in = nc.declare_dram_parameter("hpar", [128, 7 * HR], F32, False)
    cpar_in = nc.declare_dram_parameter("cpar", [128, 4], F32, False)
    out_r = nc.declare_dram_parameter("out_r", [C, T], F32, True)
    skip_r = nc.declare_dram_parameter("skip_r", [SC, T], F32, True)

    n_act = CFG["drain_act"]

    with tile.TileContext(nc) as tc:
        with (
            tc.tile_pool(name="persist", bufs=1) as pp,
            tc.tile_pool(name="outp", bufs=3) as outp,
            tc.tile_pool(name="small", bufs=1) as small,
            tc.tile_pool(name="mm", bufs=2, space="PSUM") as mmp,
        ):
            # ---- x first (8 chunks), cast on DVE
            x_t = pp.tile([128, T], F32, tag="x")
            xb = pp.tile([128, T], BF16, tag="xb")
            for cch in range(8):
                sl = slice(cch * 500, (cch + 1) * 500)
                nc.sync.dma_start(out=x_t[:, sl], in_=x_in[:, sl])
                nc.vector.tensor_copy(out=xb[:, sl], in_=x_t[:, sl])

            # ---- weights / params
            w1t = pp.tile([128, H], BF16, tag="w1t")
            nc.sync.dma_start(out=w1t[:], in_=lhsT1_in[:])
            dwd = pp.tile([128, HR * 3, 128], BF16, tag="dwd")
            nc.sync.dma_start(out=dwd[:], in_=_r3(dwdiag_in[:], 128))
            w2t = pp.tile([128, HR, 128], BF16, tag="w2t")
            nc.sync.dma_start(out=w2t[:], in_=_r3(lhsT2_in[:], 128))
            wst = pp.tile([128, HR, 128], BF16, tag="wst")
            nc.sync.dma_start(out=wst[:], in_=_r3(lhsTsk_in[:], 128))
            hpar = pp.tile([128, 7, HR], F32, tag="hpar")
            nc.sync.dma_start(out=hpar[:], in_=_r3(hpar_in[:], HR))
            cpar = pp.tile([128, 4], F32, tag="cpar")
            nc.sync.dma_start(out=cpar[:], in_=cpar_in[:])
            a1c, b1c, g1c, be1c, adwc, g2c, be2c = (hpar[:, i, :] for i in range(7))
            a2c, b2c, askc, bskc = (cpar[:, i:i + 1] for i in range(4))
            eps_t = small.tile([128, 1], F32, tag="eps")
            nc.vector.memset(eps_t[:], EPS)
            ones_c = small.tile([128, 1], F32, tag="ones_c")
            nc.vector.memset(ones_c[:], 1.0)
            ones_r = small.tile([1, 128], F32, tag="ones_r")
            nc.vector.memset(ones_r[:], 1.0)

            # ---- big persistent tensors (h1 and h2n share slots)
            x2_t = pp.tile([128, T], F32, tag="x2")
            scr_d = pp.tile([128, T], BF16, tag="scr_d")    # DVE-only scratch
            scr_a = pp.tile([128, T], BF16, tag="scr_a")    # ACT-only scratch
            h1 = [pp.tile([128, T], BF16, tag=f"hbig_{r}", name=f"h1_{r}")
                  for r in range(HR)]
            h1n = [pp.tile([128, T + 2 * DIL], BF16, tag=f"h1n_{r}", name=f"h1n_{r}")
                   for r in range(HR)]
            h2 = [pp.tile([128, T], BF16, tag=f"h2_{r}", name=f"h2_{r}")
                  for r in range(HR)]
            h2n = [pp.tile([128, T], BF16, tag=f"hbig_{r}", name=f"h2n_{r}")
                   for r in range(HR)]

            # stats partials: one tile per block; drains/sumsq write disjoint
            # columns (cols 0:8 group sums, 8:12 per-row sumsq)
            st1 = small.tile([128, 12], F32, tag="st1")
            st2 = small.tile([128, 12], F32, tag="st2")
            scb1 = small.tile([128, 2 * HR], F32, tag="scb1")
            scb2 = small.tile([128, 2 * HR], F32, tag="scb2")
            ws1 = small.tile([128, 8], F32, tag="ws1")
            ws2 = small.tile([128, 8], F32, tag="ws2")

            def drain(gidx, ps4, dst2000, scale_col, bias_col, st, width=GW):
                """Drain a 4-bank PSUM group to SBUF with per-channel affine +
                accumulated per-channel sum. gidx picks ScalarE vs DVE."""
                k = width // CW
                pview = ps4[:, 0:k, 0:CW]
                oview = _r3(dst2000)
                if gidx < n_act:
                    nc.scalar.activation(oview, pview, AFT.Identity,
                                         bias=bias_col, scale=scale_col,
                                         accum_out=st[:, gidx:gidx + 1])
                else:
                    nc.vector.tensor_scalar(out=oview, in0=pview,
                                            scalar1=scale_col, scalar2=bias_col,
                                            op0=ALU.mult, op1=ALU.add,
                                            accum_out=st[:, gidx:gidx + 1])

            # PE touches the later-phase weights early so first dw/final
            # matmuls don't need a DMA wait slot (2-wait ISA limit)
            psdum = mmp.tile([1, 4], F32, tag="mm4", name="psdum")
            nc.tensor.matmul(psdum[:, 0:2], dwd[:, 0, 0:1], dwd[:, 1, 0:2],
                             start=True, stop=True)
            nc.tensor.matmul(psdum[:, 2:4], w2t[:, 0, 0:1], wst[:, 0, 0:2],
                             start=True, stop=True)

            # ---- block 1: conv1 (sign matmuls) + drains
            for r in range(HR):
                for g in range(NG):
                    ps4 = mmp.tile([128, 4, 512], F32, tag="mm4",
                                   name=f"c1ps_{r}_{g}")
                    for c4 in range(4):
                        cc = g * 4 + c4
                        nc.tensor.matmul(ps4[:, c4, 0:CW],
                                         w1t[:, r * 128:(r + 1) * 128],
                                         xb[:, cc * CW:(cc + 1) * CW],
                                         start=True, stop=True)
                    drain(r * NG + g, ps4, h1[r][:, g * GW:(g + 1) * GW],
                          a1c[:, r:r + 1], b1c[:, r:r + 1], st1)

            # residual base x2 = x + b2 (ScalarE, fills early ACT gaps;
            # chunked so each op waits on a single x-DMA queue)
            for cch in range(8):
                sl = slice(cch * 500, (cch + 1) * 500)
                nc.scalar.activation(x2_t[:, sl], x_t[:, sl], AFT.Identity,
                                     bias=b2c, scale=1.0)

            # DVE absorbs the ACT(x2) + DMA(cpar) ticks early so the first
            # residual STT only needs its PE wait
            warm = small.tile([128, 1], F32, tag="warm")
            nc.vector.scalar_tensor_tensor(out=warm[:], in0=x2_t[:, T - 1:T],
                                           scalar=a2c, in1=x2_t[:, T - 1:T],
                                           op0=ALU.mult, op1=ALU.add)

            # sum of squares of h1 per row (split ScalarE/DVE)
            nsq = CFG["sumsq_act"]
            for r in range(HR):
                if r < nsq:
                    nc.scalar.activation(scr_a[:, 0:T], h1[r][:], AFT.Square,
                                         accum_out=st1[:, 8 + r:9 + r])
                else:
                    nc.vector.scalar_tensor_tensor(
                        out=scr_d[:, 0:T], in0=h1[r][:], scalar=0.0,
                        in1=h1[r][:], op0=ALU.bypass, op1=ALU.mult,
                        accum_out=st1[:, 8 + r:9 + r])

            def stats_join(st, scb, gcol, becol, ws, tagsfx):
                nc.vector.reduce_sum(out=ws[:, 0:1], in_=st[:, 0:8], axis=AX.X)
                nc.vector.reduce_sum(out=ws[:, 1:2], in_=st[:, 8:12], axis=AX.X)
                ps_t = mmp.tile([1, 2], F32, tag="mm4", name=f"pst_{tagsfx}")
                nc.tensor.matmul(ps_t[:], ones_c[:], ws[:, 0:2],
                                 start=True, stop=True)
                st_s = small.tile([1, 2], F32, tag=f"sts_{tagsfx}")
                nc.vector.tensor_copy(out=st_s[:], in_=ps_t[:])
                ps_b = mmp.tile([128, 2], F32, tag="mm4", name=f"psb_{tagsfx}")
                nc.tensor.matmul(ps_b[:], ones_r[:], st_s[:],
                                 start=True, stop=True)
                nc.vector.tensor_copy(out=ws[:, 0:2], in_=ps_b[:])
                # m=S/N; E2=Q/N; mneg=-m; var=E2-m^2; rs=1/sqrt(var+eps)
                nc.scalar.activation(ws[:, 2:4], ws[:, 0:2], AFT.Identity,
                                     scale=1.0 / NTOT)
                nc.scalar.activation(ws[:, 4:5], ws[:, 0:1], AFT.Identity,
                                     scale=-1.0 / NTOT)
                nc.vector.tensor_tensor(out=ws[:, 5:6], in0=ws[:, 2:3],
                                        in1=ws[:, 2:3], op=ALU.mult)
                nc.vector.tensor_tensor(out=ws[:, 5:6], in0=ws[:, 3:4],
                                        in1=ws[:, 5:6], op=ALU.subtract)
                nc.scalar.activation(ws[:, 6:7], ws[:, 5:6], AFT.Sqrt,
                                     bias=eps_t[:, 0:1])
                nc.vector.reciprocal(ws[:, 7:8], ws[:, 6:7])
                nc.vector.tensor_scalar_mul(out=scb[:, 0:HR], in0=gcol,
                                            scalar1=ws[:, 7:8])
                nc.vector.scalar_tensor_tensor(out=scb[:, HR:2 * HR],
                                               in0=scb[:, 0:HR],
                                               scalar=ws[:, 4:5], in1=becol,
                                               op0=ALU.mult, op1=ALU.add)

            stats_join(st1, scb1, g1c, be1c, ws1, "1")

            # ---- np1: prelu(scale*h1+bias) -> h1n (halo-padded)
            def np_unit(uidx, n_on_act, src, dst, sc_col, bi_col, pval):
                if uidx < n_on_act:
                    nc.scalar.activation(dst, src, AFT.Prelu,
                                         bias=bi_col, scale=sc_col, alpha=pval)
                else:
                    w = src.shape[-1]
                    z = scr_d[:, 0:w]
                    nc.vector.tensor_scalar(out=z, in0=src, scalar1=sc_col,
                                            scalar2=bi_col,
                                            op0=ALU.mult, op1=ALU.add)
                    nc.vector.scalar_tensor_tensor(out=dst, in0=z, scalar=pval,
                                                   in1=z, op0=ALU.mult,
                                                   op1=ALU.max)

            np1_act_rows = CFG["np1_act_rows"]
            for r in range(HR):
                on_act = r < np1_act_rows
                if on_act:
                    nc.scalar.activation(h1n[r][:, 0:DIL], hpar[:, 0, 0:DIL],
                                         AFT.Identity, bias=0.0, scale=0.0)
                    nc.scalar.activation(h1n[r][:, DIL + T:], hpar[:, 0, 0:DIL],
                                         AFT.Identity, bias=0.0, scale=0.0)
                else:
                    nc.vector.memset(h1n[r][:, 0:DIL], 0)
                    nc.vector.memset(h1n[r][:, DIL + T:], 0)
                for g in range(NG):
                    sl = slice(g * 2000, (g + 1) * 2000)
                    np_unit(0 if on_act else 1, 1,
                            h1[r][:, sl],
                            h1n[r][:, DIL + g * 2000:DIL + (g + 1) * 2000],
                            scb1[:, r:r + 1], scb1[:, HR + r:HR + r + 1], p1)

            # ---- block 2: depthwise dilated conv (diag sign matmuls)
            for r in range(HR):
                for g in range(NG):
                    ps4 = mmp.tile([128, 4, 512], F32, tag="mm4",
                                   name=f"dwps_{r}_{g}")
                    for k in range(3):
                        off = (k - 1) * DIL
                        for c4 in range(4):
                            cc = g * 4 + c4
                            st_ = DIL + cc * CW + off
                            nc.tensor.matmul(ps4[:, c4, 0:CW],
                                             dwd[:, r * 3 + k, :],
                                             h1n[r][:, st_:st_ + CW],
                                             start=(k == 0), stop=(k == 2))
                    drain(r * NG + g, ps4, h2[r][:, g * GW:(g + 1) * GW],
                          adwc[:, r:r + 1], 0.0, st2)

            for r in range(HR):
                if r < nsq:
                    nc.scalar.activation(scr_a[:, 0:T], h2[r][:], AFT.Square,
                                         accum_out=st2[:, 8 + r:9 + r])
                else:
                    nc.vector.scalar_tensor_tensor(
                        out=scr_d[:, 0:T], in0=h2[r][:], scalar=0.0,
                        in1=h2[r][:], op0=ALU.bypass, op1=ALU.mult,
                        accum_out=st2[:, 8 + r:9 + r])

            stats_join(st2, scb2, g2c, be2c, ws2, "2")

            # ---- np2 (pair-major so the final matmuls can start early)
            def np2_on_act(pr, r):
                # ~10/16 units on ScalarE; row 3 always DVE (and row 2 for
                # late pairs) so every pair has a DVE-written row
                return r <= 1 or (r == 2 and pr < 2)

            for pr in range(4):
                sl = slice(pr * 1000, (pr + 1) * 1000)
                for r in range(HR):
                    np_unit(0 if np2_on_act(pr, r) else 1, 1,
                            h2[r][:, sl], h2n[r][:, sl],
                            scb2[:, r:r + 1], scb2[:, HR + r:HR + r + 1], p2)

            # ---- finals: out (+residual) and skip per 1000-wide pair
            for pr in range(4):
                ps4 = mmp.tile([128, 4, 512], F32, tag="mm4", name=f"fin_{pr}")
                korder = [3, 0, 1, 2]  # start on the DVE-written row
                for ki, k in enumerate(korder):
                    for j in range(2):
                        cc = pr * 2 + j
                        csl = slice(cc * CW, (cc + 1) * CW)
                        nc.tensor.matmul(ps4[:, j, 0:CW], w2t[:, k, :],
                                         h2n[k][:, csl],
                                         start=(ki == 0), stop=(ki == HR - 1))
                for ki, k in enumerate(korder):
                    for j in range(2):
                        cc = pr * 2 + j
                        csl = slice(cc * CW, (cc + 1) * CW)
                        nc.tensor.matmul(ps4[:, 2 + j, 0:CW], wst[:, k, :],
                                         h2n[k][:, csl],
                                         start=(ki == 0), stop=(ki == HR - 1))
                psl = slice(pr * 1000, (pr + 1) * 1000)
                oc = outp.tile([128, 1000], F32, tag="oc")
                nc.vector.scalar_tensor_tensor(out=_r3(oc[:]),
                                               in0=ps4[:, 0:2, 0:CW],
                                               scalar=a2c,
                                               in1=_r3(x2_t[:, psl]),
                                               op0=ALU.mult, op1=ALU.add)
                nc.sync.dma_start(out=out_r[:, psl], in_=oc[:])
                sc = outp.tile([128, 1000], F32, tag="sc")
                nc.vector.tensor_scalar(out=_r3(sc[:]), in0=ps4[:, 2:4, 0:CW],
                                        scalar1=askc, scalar2=bskc,
                                        op0=ALU.mult, op1=ALU.add)
                nc.sync.dma_start(out=skip_r[:, psl], in_=sc[:])
    return nc


def _install_ntff_hook():
    """The agent image's antenv lacks axon_hooks; recreate it from the boot
    helper so run_bass_kernel_spmd(trace=True) can capture NTFF profiles."""
    import types
    try:
        from antenv.axon_hooks import get_axon_ntff_profile_hook  # noqa: F401
        return
    except ImportError:
        pass
    try:
        if "/root/.axon_site" not in sys.path:
            sys.path.insert(0, "/root/.axon_site")
        from trn_agent_boot.trn_boot import _ntff_profile_via_ctypes
        hook = _ntff_profile_via_ctypes("/opt/axon/libaxon_pjrt.so")
    except Exception:
        hook = None
    mod = types.ModuleType("antenv.axon_hooks")
    state = {"hook": hook}
    mod.get_axon_ntff_profile_hook = lambda: state["hook"]
    mod.set_axon_ntff_profile_hook = lambda h: state.update(hook=h)
    sys.modules["antenv.axon_hooks"] = mod


def kernel(**inputs):
    x, p1, p2, common = _prep(inputs)
    nc = _build(p1, p2)
    if not nc.is_finalized():
        nc.finalize()
    in_maps = [dict(common, x_in=np.ascontiguousarray(x[b])) for b in range(B)]
    trace = bool(int(os.environ.get("KERNEL_TRACE", "0")))
    if trace:
        _install_ntff_hook()
    res = run_bass_kernel_spmd(nc, in_maps, core_ids=list(range(B)), trace=trace)
    last_run_info.clear()
    last_run_info["exec_time_ns"] = res.exec_time_ns
    last_run_info["results"] = res
    out = np.stack([r["out_r"] for r in res.results]).astype(np.float32)
    skip = np.stack([r["skip_r"] for r in res.results]).astype(np.float32)
    return out, skip


# revision 30
# speedup vs baseline: 1.9479x; 1.1441x over previous
"""Trainium2 Bass kernel for nn_BinaryTemporalBlock (Conv-TasNet-style binary
temporal block): 1x1 binarized conv (128->512) -> gLN -> PReLU -> dilated
depthwise binarized conv (K=3, dil=4) -> gLN -> PReLU -> two 1x1 binarized
convs (512->128 residual-out and 512->128 skip).

Sharding: data-parallel over batch. B=8 samples on 8 NeuronCores, one sample
per core; gLN is per-sample so no collectives are needed.

Per-core strategy (sample = [C=128, T=4000]):
  - Host binarizes weights: sign matrices (exact +-1 in bf16) go through the
    PE array; per-output-channel alpha scales stay fp32 and ride the
    PSUM->SBUF drains (free affine in ScalarE activation / DVE tensor_scalar).
  - Matmuls fill 4-bank PSUM groups ([128,4,512]); one drain per group with
    accum_out collecting per-channel sums for gLN. Sum-of-squares by a second
    pass (ScalarE Square+accum or DVE scalar_tensor_tensor+accum), engine
    split tuned via CFG.
  - Partition reduce+broadcast of stats via two tiny PE matmuls with ones.
  - norm+PReLU: ScalarE Prelu (scale/bias/alpha) on half the tiles, DVE
    tensor_scalar + max(z, p*z) on the rest (valid for p <= 1).
  - depthwise dilated conv: 3 diagonal-sign matmuls per tile accumulating in
    PSUM (taps at t-4, t, t+4 via shifted APs on a halo-padded tile).
"""

import os
import sys

sys.path.insert(0, "/opt/trn_rl_repo")

import numpy as np
import ml_dtypes

import concourse.bass as bass
import concourse.tile as tile
from concourse import bacc
from concourse import mybir
from concourse.bass_utils import run_bass_kernel_spmd

F32 = mybir.dt.float32
BF16 = mybir.dt.bfloat16
NPBF16 = ml_dtypes.bfloat16
ALU = mybir.AluOpType
AFT = mybir.ActivationFunctionType
AX = mybir.AxisListType

B, C, H, SC, T = 8, 128, 512, 128, 4000
HR = H // 128          # 4 h-rows of 128 partitions
CW = 500               # matmul chunk width (<=512 fp32 PSUM bank)
GB = 2                 # PSUM banks per drain group
GW = GB * CW           # drain-group width
NG = T // GW           # groups per row
DIL = 4
EPS = 1e-8
NTOT = float(H * T)

CFG = {
    "np1_act_rows": (0, 1),      # h1n rows normalized on ScalarE Prelu
    "np2_act_rows": (0, 1),      # h2n rows on ScalarE Prelu
    "sumsq_act_rows": (1, 3),    # rows whose sum-of-squares runs on ScalarE
    "skip_act_pairs": (0, 1),    # skip-drain pairs on ScalarE
}

last_run_info = {}


def _binarize(w):
    alpha = np.mean(np.abs(w), axis=tuple(range(1, w.ndim)))
    return alpha.astype(np.float32), np.sign(w).astype(np.float32)


def _cols(v):
    """[512] channel vector -> [128, HR] column-per-h-row layout."""
    return np.ascontiguousarray(v.reshape(HR, 128).T.astype(np.float32))


def _prep(inputs):
    x = np.asarray(inputs["x"], np.float32)
    p1 = float(np.asarray(inputs["p1"]))
    p2 = float(np.asarray(inputs["p2"]))
    b1 = np.asarray(inputs["b1"], np.float32).reshape(-1)
    g1 = np.asarray(inputs["g1"], np.float32).reshape(-1)
    be1 = np.asarray(inputs["be1"], np.float32).reshape(-1)
    g2 = np.asarray(inputs["g2"], np.float32).reshape(-1)
    be2 = np.asarray(inputs["be2"], np.float32).reshape(-1)
    b2 = np.asarray(inputs["b2"], np.float32).reshape(-1)
    bsk = np.asarray(inputs["b_skip"], np.float32).reshape(-1)

    a1, s1 = _binarize(np.asarray(inputs["w1"], np.float32))
    adw, sdw = _binarize(np.asarray(inputs["w_dw"], np.float32))
    a2, s2 = _binarize(np.asarray(inputs["w2"], np.float32))
    ask, ssk = _binarize(np.asarray(inputs["w_skip"], np.float32))
    s1 = s1[:, :, 0]      # [512,128]
    sdw = sdw[:, 0, :]    # [512,3]
    s2 = s2[:, :, 0]      # [128,512]
    ssk = ssk[:, :, 0]

    # one packed bf16 weight tensor: lhsT1 | dwdiag | lhsT2 | lhsTsk
    wcat = np.zeros((128, 24, 128), NPBF16)
    wcat[:, 0:4, :] = s1.T.reshape(128, 4, 128)
    for r in range(HR):
        for k in range(3):
            np.fill_diagonal(wcat[:, 4 + r * 3 + k, :],
                             sdw[r * 128:(r + 1) * 128, k])
    for k in range(HR):
        wcat[:, 16 + k, :] = s2[:, k * 128:(k + 1) * 128].T
        wcat[:, 20 + k, :] = ssk[:, k * 128:(k + 1) * 128].T

    # one packed fp32 param tensor: 7 h-row columns [128,7,4] + 4 C columns
    fpar = np.zeros((128, 32), np.float32)
    hcols = np.stack([_cols(a1), _cols(b1), _cols(g1), _cols(be1),
                      _cols(adw), _cols(g2), _cols(be2)], axis=1)  # [128,7,4]
    fpar[:, 0:28] = hcols.reshape(128, 28)
    fpar[:, 28:32] = np.stack([a2, b2, ask, bsk], axis=1)

    common = {
        "wcat": np.ascontiguousarray(wcat.reshape(128, 24 * 128)),
        "fpar": np.ascontiguousarray(fpar),
    }
    return x, p1, p2, common


def _r3(ap, b=CW):
    """[128, k*b] contiguous slice -> [128, k, b] view."""
    return ap.rearrange("p (a b) -> p a b", b=b)


def _build(p1, p2):
    nc = bacc.Bacc("TRN2", target_bir_lowering=False, debug=False, num_devices=8)
    x_in = nc.declare_dram_parameter("x_in", [C, T], F32, False)
    wcat_in = nc.declare_dram_parameter("wcat", [128, 24 * 128], BF16, False)
    fpar_in = nc.declare_dram_parameter("fpar", [128, 32], F32, False)
    out_r = nc.declare_dram_parameter("out_r", [C, T], F32, True)
    skip_r = nc.declare_dram_parameter("skip_r", [SC, T], F32, True)

    np1_act = CFG["np1_act_rows"]
    np2_act = CFG["np2_act_rows"]
    sq_act = CFG["sumsq_act_rows"]
    skip_act = CFG["skip_act_pairs"]

    with tile.TileContext(nc) as tc:
        with (
            tc.tile_pool(name="persist", bufs=1) as pp,
            tc.tile_pool(name="outp", bufs=3) as outp,
            tc.tile_pool(name="small", bufs=1) as small,
            tc.tile_pool(name="mm", bufs=4, space="PSUM") as mmp,
        ):
            # ---- x first (4 chunks), cast to bf16 on DVE
            x_t = pp.tile([128, T], F32, tag="x")
            xb = pp.tile([128, T], BF16, tag="xb")
            for q in range(4):
                sl = slice(q * 1000, (q + 1) * 1000)
                nc.sync.dma_start(out=x_t[:, sl], in_=x_in[:, sl])
                nc.vector.tensor_copy(out=xb[:, sl], in_=x_t[:, sl])

            # ---- packed weights / params (one DMA each)
            wcat = pp.tile([128, 24, 128], BF16, tag="wcat")
            nc.sync.dma_start(out=wcat[:], in_=_r3(wcat_in[:], 128))
            fpar = pp.tile([128, 32], F32, tag="fpar")
            nc.sync.dma_start(out=fpar[:], in_=fpar_in[:])
            hp = _r3(fpar[:, 0:28], HR)                   # [128,7,4]
            a1c, b1c, g1c, be1c, adwc, g2c, be2c = (hp[:, i, :] for i in range(7))
            a2c, b2c, askc, bskc = (fpar[:, 28 + i:29 + i] for i in range(4))
            w1s = wcat[:, 0:4, :]
            dws = wcat[:, 4:16, :]
            w2s = wcat[:, 16:20, :]
            wss = wcat[:, 20:24, :]
            eps_t = small.tile([128, 1], F32, tag="eps")
            nc.vector.memset(eps_t[:], EPS)
            ones_c = small.tile([128, 1], F32, tag="ones_c")
            nc.vector.memset(ones_c[:], 1.0)
            ones_r = small.tile([1, 128], F32, tag="ones_r")
            nc.vector.memset(ones_r[:], 1.0)

            # residual base x2 = x + b2 on DVE (also absorbs the fpar DMA
            # tick on DVE before the drains need it)
            x2_t = pp.tile([128, T], F32, tag="x2")
            for q in range(4):
                sl = slice(q * 1000, (q + 1) * 1000)
                nc.vector.tensor_scalar(out=x2_t[:, sl], in0=x_t[:, sl],
                                        scalar1=b2c, scalar2=None, op0=ALU.add)

            scr_d = pp.tile([128, T], BF16, tag="scr_d")    # DVE-only scratch
            scr_a = pp.tile([128, T], BF16, tag="scr_a")    # ACT-only scratch
            h1 = [pp.tile([128, T], BF16, tag=f"hbig_{r}", name=f"h1_{r}")
                  for r in range(HR)]
            h1n = [pp.tile([128, T + 2 * DIL], BF16, tag=f"h1n_{r}", name=f"h1n_{r}")
                   for r in range(HR)]
            h2 = [pp.tile([128, T], BF16, tag=f"h2_{r}", name=f"h2_{r}")
                  for r in range(HR)]
            h2n = [pp.tile([128, T], BF16, tag=f"hbig_{r}", name=f"h2n_{r}")
                   for r in range(HR)]

            st1 = small.tile([128, 2 * HR * NG + HR], F32, tag="st1")
            st2 = small.tile([128, 2 * HR * NG + HR], F32, tag="st2")
            scb1 = small.tile([128, 2 * HR], F32, tag="scb1")
            scb2 = small.tile([128, 2 * HR], F32, tag="scb2")
            ws1 = small.tile([128, 8], F32, tag="ws1")
            ws2 = small.tile([128, 8], F32, tag="ws2")

            def drain(gidx, ps4, dst2000, scale_col, bias_col, st):
                """Empty a 4-bank PSUM group with per-channel affine and
                accumulate per-channel sums. Engine by gidx parity (aligned
                with the 2-slot PSUM cycle so slot WAW stays same-engine)."""
                pview = ps4[:, 0:GB, 0:CW]
                oview = _r3(dst2000)
                if gidx % 2 == 0:
                    nc.scalar.activation(oview, pview, AFT.Identity,
                                         bias=bias_col, scale=scale_col,
                                         accum_out=st[:, gidx:gidx + 1])
                else:
                    nc.vector.tensor_scalar(out=oview, in0=pview,
                                            scalar1=scale_col, scalar2=bias_col,
                                            op0=ALU.mult, op1=ALU.add,
                                            accum_out=st[:, gidx:gidx + 1])

            def sumsq(r, src, st):
                if r in sq_act:
                    nc.scalar.activation(scr_a[:, 0:T], src, AFT.Square,
                                         accum_out=st[:, 2 * HR * NG + r:2 * HR * NG + r + 1])
                else:
                    nc.vector.scalar_tensor_tensor(
                        out=scr_d[:, 0:T], in0=src, scalar=0.0, in1=src,
                        op0=ALU.bypass, op1=ALU.mult,
                        accum_out=st[:, 2 * HR * NG + r:2 * HR * NG + r + 1])

            # ---- block 1: conv1 (sign matmuls) + drains
            for r in range(HR):
                for g in range(NG):
                    ps4 = mmp.tile([128, GB, 512], F32, tag="mm4",
                                   name=f"c1ps_{r}_{g}")
                    for c4 in range(GB):
                        cc = g * GB + c4
                        nc.tensor.matmul(ps4[:, c4, 0:CW], w1s[:, r, :],
                                         xb[:, cc * CW:(cc + 1) * CW],
                                         start=True, stop=True)
                    drain(r * NG + g, ps4, h1[r][:, g * GW:(g + 1) * GW],
                          a1c[:, r:r + 1], b1c[:, r:r + 1], st1)
            for r in range(HR):
                sumsq(r, h1[r][:], st1)

            def stats_join(st, scb, gcol, becol, ws, tagsfx):
                nsum = 2 * HR * NG
                nc.vector.reduce_sum(out=ws[:, 0:1], in_=st[:, 0:nsum], axis=AX.X)
                nc.vector.reduce_sum(out=ws[:, 1:2], in_=st[:, nsum:nsum + HR],
                                     axis=AX.X)
                ps_t = mmp.tile([1, 2], F32, tag="mm4", name=f"pst_{tagsfx}")
                nc.tensor.matmul(ps_t[:], ones_c[:], ws[:, 0:2],
                                 start=True, stop=True)
                st_s = small.tile([1, 2], F32, tag=f"sts_{tagsfx}")
                nc.vector.tensor_copy(out=st_s[:], in_=ps_t[:])
                ps_b = mmp.tile([128, 2], F32, tag="mm4", name=f"psb_{tagsfx}")
                nc.tensor.matmul(ps_b[:], ones_r[:], st_s[:],
                                 start=True, stop=True)
                nc.vector.tensor_copy(out=ws[:, 0:2], in_=ps_b[:])
                nc.scalar.activation(ws[:, 2:4], ws[:, 0:2], AFT.Identity,
                                     scale=1.0 / NTOT)
                nc.scalar.activation(ws[:, 4:5], ws[:, 0:1], AFT.Identity,
                                     scale=-1.0 / NTOT)
                nc.vector.tensor_tensor(out=ws[:, 5:6], in0=ws[:, 2:3],
                                        in1=ws[:, 2:3], op=ALU.mult)
                nc.vector.tensor_tensor(out=ws[:, 5:6], in0=ws[:, 3:4],
                                        in1=ws[:, 5:6], op=ALU.subtract)
                nc.scalar.activation(ws[:, 6:7], ws[:, 5:6], AFT.Sqrt,
                                     bias=eps_t[:, 0:1])
                nc.vector.reciprocal(ws[:, 7:8], ws[:, 6:7])
                nc.vector.tensor_scalar_mul(out=scb[:, 0:HR], in0=gcol,
                                            scalar1=ws[:, 7:8])
                nc.vector.scalar_tensor_tensor(out=scb[:, HR:2 * HR],
                                               in0=scb[:, 0:HR],
                                               scalar=ws[:, 4:5], in1=becol,
                                               op0=ALU.mult, op1=ALU.add)

            stats_join(st1, scb1, g1c, be1c, ws1, "1")

            # ---- np: prelu(scale*h+bias); ScalarE Prelu or DVE 3-op form
            def np_unit(on_act, src, dst, sc_col, bi_col, pval):
                if on_act:
                    nc.scalar.activation(dst, src, AFT.Prelu,
                                         bias=bi_col, scale=sc_col, alpha=pval)
                else:
                    w = src.shape[-1]
                    z = scr_d[:, 0:w]
                    pz = scr_d[:, w:2 * w]
                    nc.vector.tensor_scalar(out=z, in0=src, scalar1=sc_col,
                                            scalar2=bi_col,
                                            op0=ALU.mult, op1=ALU.add)
                    nc.vector.tensor_scalar_mul(out=pz, in0=z, scalar1=pval)
                    nc.vector.tensor_tensor(out=dst, in0=z, in1=pz, op=ALU.max)

            for r in range(HR):
                on_act = r in np1_act
                if on_act:
                    nc.scalar.activation(h1n[r][:, 0:DIL], fpar[:, 0:DIL],
                                         AFT.Identity, bias=0.0, scale=0.0)
                    nc.scalar.activation(h1n[r][:, DIL + T:], fpar[:, 0:DIL],
                                         AFT.Identity, bias=0.0, scale=0.0)
                else:
                    nc.vector.memset(h1n[r][:, 0:DIL], 0)
                    nc.vector.memset(h1n[r][:, DIL + T:], 0)
                for u in range(T // 2000):
                    sl = slice(u * 2000, (u + 1) * 2000)
                    np_unit(on_act, h1[r][:, sl],
                            h1n[r][:, DIL + u * 2000:DIL + (u + 1) * 2000],
                            scb1[:, r:r + 1], scb1[:, HR + r:HR + r + 1], p1)

            # ---- block 2: depthwise dilated conv (diag sign matmuls)
            for r in range(HR):
                for g in range(NG):
                    ps4 = mmp.tile([128, GB, 512], F32, tag="mm4",
                                   name=f"dwps_{r}_{g}")
                    for k in range(3):
                        off = (k - 1) * DIL
                        for c4 in range(GB):
                            cc = g * GB + c4
                            st_ = DIL + cc * CW + off
                            nc.tensor.matmul(ps4[:, c4, 0:CW],
                                             dws[:, r * 3 + k, :],
                                             h1n[r][:, st_:st_ + CW],
                                             start=(k == 0), stop=(k == 2))
                    drain(r * NG + g, ps4, h2[r][:, g * GW:(g + 1) * GW],
                          adwc[:, r:r + 1], 0.0, st2)
            for r in range(HR):
                sumsq(r, h2[r][:], st2)

            stats_join(st2, scb2, g2c, be2c, ws2, "2")

            # ---- np2 (pair-major so the final matmuls start early)
            for pr in range(4):
                sl = slice(pr * 1000, (pr + 1) * 1000)
                for r in range(HR):
                    np_unit(r in np2_act, h2[r][:, sl], h2n[r][:, sl],
                            scb2[:, r:r + 1], scb2[:, HR + r:HR + r + 1], p2)

            # ---- finals: out (+residual) and skip per 1000-wide pair
            for pr in range(4):
                ps_o = mmp.tile([128, GB, 512], F32, tag="mm4", name=f"fino_{pr}")
                ps_s = mmp.tile([128, GB, 512], F32, tag="mm4", name=f"fins_{pr}")
                korder = [3, 0, 1, 2]  # start on a DVE-written h2n row
                for ki, k in enumerate(korder):
                    for j in range(2):
                        cc = pr * 2 + j
                        csl = slice(cc * CW, (cc + 1) * CW)
                        nc.tensor.matmul(ps_o[:, j, 0:CW], w2s[:, k, :],
                                         h2n[k][:, csl],
                                         start=(ki == 0), stop=(ki == HR - 1))
                for ki, k in enumerate(korder):
                    for j in range(2):
                        cc = pr * 2 + j
                        csl = slice(cc * CW, (cc + 1) * CW)
                        nc.tensor.matmul(ps_s[:, j, 0:CW], wss[:, k, :],
                                         h2n[k][:, csl],
                                         start=(ki == 0), stop=(ki == HR - 1))
                psl = slice(pr * 1000, (pr + 1) * 1000)
                oc = outp.tile([128, 1000], F32, tag="oc")
                nc.vector.scalar_tensor_tensor(out=_r3(oc[:]),
                                               in0=ps_o[:, 0:GB, 0:CW],
                                               scalar=a2c,
                                               in1=_r3(x2_t[:, psl]),
                                               op0=ALU.mult, op1=ALU.add)
                nc.sync.dma_start(out=out_r[:, psl], in_=oc[:])
                sc = outp.tile([128, 1000], F32, tag="sc")
                if pr in skip_act:
                    nc.scalar.activation(_r3(sc[:]), ps_s[:, 0:GB, 0:CW],
                                         AFT.Identity, bias=bskc, scale=askc)
                else:
                    nc.vector.tensor_scalar(out=_r3(sc[:]),
                                            in0=ps_s[:, 0:GB, 0:CW],
                                            scalar1=askc, scalar2=bskc,
                                            op0=ALU.mult, op1=ALU.add)
                nc.sync.dma_start(out=skip_r[:, psl], in_=sc[:])
    return nc


def _install_ntff_hook():
    """The agent image's antenv lacks axon_hooks; recreate it from the boot
    helper so run_bass_kernel_spmd(trace=True) can capture NTFF profiles."""
    import types
    try:
        from antenv.axon_hooks import get_axon_ntff_profile_hook  # noqa: F401
        return
    except ImportError:
        pass
    try:
        if "/root/.axon_site" not in sys.path:
            sys.path.insert(0, "/root/.axon_site")
        from trn_agent_boot.trn_boot import _ntff_profile_via_ctypes
        hook = _ntff_profile_via_ctypes("/opt/axon/libaxon_pjrt.so")
    except Exception:
        hook = None
    mod = types.ModuleType("antenv.axon_hooks")
    state = {"hook": hook}
    mod.get_axon_ntff_profile_hook = lambda: state["hook"]
    mod.set_axon_ntff_profile_hook = lambda h: state.update(hook=h)
    sys.modules["antenv.axon_hooks"] = mod


def kernel(**inputs):
    x, p1, p2, common = _prep(inputs)
    nc = _build(p1, p2)
    if not nc.is_finalized():
        nc.finalize()
    in_maps = [dict(common, x_in=np.ascontiguousarray(x[b])) for b in range(B)]
    trace = bool(int(os.environ.get("KERNEL_TRACE", "0")))
    if trace:
        _install_ntff_hook()
    res = run_bass_kernel_spmd(nc, in_maps, core_ids=list(range(B)), trace=trace)
    last_run_info.clear()
    last_run_info["exec_time_ns"] = res.exec_time_ns
    last_run_info["results"] = res
    out = np.stack([r["out_r"] for r in res.results]).astype(np.float32)
    skip = np.stack([r["skip_r"] for r in res.results]).astype(np.float32)
    return out, skip
